# revision 61
# speedup vs baseline: 1.3629x; 1.0001x over previous
"""Trainium2 Bass kernel for nn_EventSplitter (edge-restricted graph transformer).

kernel(**inputs) takes the FULL unsharded numpy inputs (as produced by
reference.setup_inputs()) and returns the FULL [E, 1] float32 output.

Sharding (8 NeuronCores, one SPMD program):
  - Nodes padded to n_cores * B * 128; core c owns nodes [c*B*128, (c+1)*B*128).
  - Edges assigned to the core owning their dst, sorted by dst; every
    (core, dst-block) run padded to K chunks of 128 edges (compile-time
    constant structure shared by all cores).
  - Residual stream h kept per-core in SBUF as f16 [128, B, 192].
  - Per layer: q||qwk and [k|v] computed via PE; [k|v] f16 AllGathered into a
    shared DRAM table; edges processed in W=4 chunk batches with one W=8
    indirect row-gather per two batches.  Edge bias e = edge_attr @ We enters
    the logits through the host-precomputed rowdot matrix M_k = Wq_s We^T
    (per head) and enters the messages through a block-diagonal We matmul
    applied to the segment-summed a*ea moments - no per-edge e
    materialization at all.
  - Softmax skips max-subtraction (logits are O(1)); den division applied at
    the node level, matching the reference within 2e-2.
  - Edge head computed fully transposed: z1^T accumulates w-part (one-hot
    matmul), ea-part, and u^T (gather + identity-matmul transpose) in PSUM;
    z2/z3 are plain matmuls; biases folded via ones-rows; z3 DMA'd straight
    from PSUM.
"""

import math

import numpy as np

import concourse.bass as bass
import concourse.tile as tile
import concourse.mybir as mybir
from concourse.masks import make_identity

# --- walrus sync-command workaround (inlined) ---

_ctr = [0]

_ZERO_SYNC_TYPES = ("InstIota",)


def _mk_nop(engine, waits, updates):
    nop = mybir.InstNoOp(name=f"wsplit-{_ctr[0]}", ins=[], outs=[])
    _ctr[0] += 1
    nop.engine = engine
    nop.sync_info = mybir.SyncInfo(on_wait=list(waits), on_update=list(updates))
    return nop


def split_excess_waits(nc, max_waits=1):
    for f in nc.m.functions:
        for bb in f.blocks:
            out = []
            changed = False
            for ins in bb.instructions:
                si = ins.sync_info
                zero_sync = type(ins).__name__ in _ZERO_SYNC_TYPES
                if si is None:
                    out.append(ins)
                    continue
                waits = list(si.on_wait)
                updates = list(si.on_update)
                limit = 0 if zero_sync else max_waits
                post_updates = updates if zero_sync and updates else []
                if len(waits) > limit or post_updates:
                    keep_w = waits[len(waits) - limit:] if limit else []
                    extra_w = waits[:len(waits) - limit] if limit else waits
                    step = max(1, max_waits)
                    for i in range(0, len(extra_w), step):
                        out.append(_mk_nop(ins.engine, extra_w[i:i + step], []))
                    ins.sync_info = mybir.SyncInfo(
                        on_wait=list(keep_w),
                        on_update=[] if post_updates else list(updates))
                    out.append(ins)
                    if post_updates:
                        out.append(_mk_nop(ins.engine, [], post_updates))
                    changed = True
                else:
                    out.append(ins)
            if changed:
                bb.instructions[:] = out
    return nc


P = 128
dt = mybir.dt
f32, f16 = np.float32, np.float16

HID = 192
HEADS = 4
DH = 48
L = 3
FFN = 384
EA = 4
GP = 3
SP_ = 3
EP = 6
G_TBL = 512
XS = 7          # x(4) + splitter_probs(3)
H2 = 2 * HID    # 384
Z2 = 96
QW = HID + HEADS * EA   # 208: q(192) || qwk(16)
ACC = HEADS + HEADS * EA + HID   # 212: den(4) | T(16) | msgv(192)


# ----------------------------------------------------------------------------
# host-side sharding / index prep
# ----------------------------------------------------------------------------

def _host_prep(x, edge_index, edge_attr, batch, group_ptr, time_group_ids,
               group_probs, splitter_probs, endpoint_preds, n_cores):
    N = x.shape[0]
    E = edge_index.shape[1]
    B = int(math.ceil(N / (n_cores * P)))     # dst-blocks per core
    NLp = B * P                               # local nodes per core (padded)
    Np = NLp * n_cores

    src0 = np.asarray(edge_index[0], np.int64)
    dst0 = np.asarray(edge_index[1], np.int64)

    gids0 = np.clip(np.asarray(group_ptr)[np.asarray(batch)] + np.asarray(time_group_ids),
                    0, group_probs.shape[0] - 1).astype(np.int64)

    n_blocks_total = Np // P
    # degree-balanced node -> (block, slot) relabeling: equalizes incoming
    # edges per 128-node dst-block so K (chunks per block) shrinks.
    import heapq
    deg = np.bincount(dst0, minlength=N)
    heap = [(0, 0, b) for b in range(n_blocks_total)]
    heapq.heapify(heap)
    new_of_old = np.empty(N, np.int64)
    fill = np.zeros(n_blocks_total, np.int64)
    for old in np.argsort(-deg, kind="stable"):
        while True:
            load, f, b = heapq.heappop(heap)
            if f < P and f == fill[b]:
                break
        new_of_old[old] = b * P + f
        fill[b] = f + 1
        if f + 1 < P:
            heapq.heappush(heap, (load + int(deg[old]), f + 1, b))
    src = new_of_old[src0]
    dst = new_of_old[dst0]
    old_of_new = np.full(Np, -1, np.int64)
    old_of_new[new_of_old] = np.arange(N)

    blk_of_edge = dst // P
    order = np.argsort(dst, kind="stable")
    cnt = np.bincount(blk_of_edge, minlength=n_blocks_total)
    # chunk 0 of each block is reserved for up to 128 local-src edges
    # (gathered from kv_loc before the AllGather lands); the rest overflow
    # into the remote chunks 1..K-1.
    core_of_blk = np.arange(n_blocks_total) // B
    is_loc_e = (src // NLp) == core_of_blk[blk_of_edge]
    nloc = np.bincount(blk_of_edge[is_loc_e], minlength=n_blocks_total)
    remote_cnt = cnt - np.minimum(nloc, P)
    K = max(2, 1 + int(math.ceil(remote_cnt.max() / P)))
    C = B * K
    Ep = C * P

    sorted_eids = order
    sorted_blk = blk_of_edge[order]
    startpos = np.zeros(n_blocks_total + 1, np.int64)
    np.cumsum(cnt, out=startpos[1:])
    pos_in_blk = np.arange(E) - startpos[sorted_blk]

    core_of = sorted_blk // B
    blk_local = sorted_blk % B
    slot = blk_local * (K * P) + pos_in_blk

    src_sh = np.zeros((n_cores, Ep), np.int32)
    dstc_sh = np.zeros((n_cores, Ep), np.float16)
    amask_sh = np.zeros((n_cores, Ep), np.float16)
    ea_sh = np.zeros((n_cores, Ep, EA), np.float16)
    eid_sh = np.full((n_cores, Ep), -1, np.int64)

    ea16 = np.asarray(edge_attr, np.float16)
    for c in range(n_cores):
        m = core_of == c
        s = slot[m]
        eids = sorted_eids[m]
        src_sh[c, s] = src[eids]
        dstc_sh[c, s] = (dst[eids] % P).astype(np.float16)
        amask_sh[c, s] = 1.0
        ea_sh[c, s] = ea16[eids]
        eid_sh[c, s] = eids
    # within each (core, block): chunk 0 <- up to 128 local-src edges (pad
    # with masked slots), then remaining edges sorted by src so high-table
    # rows cluster in the trailing chunks, then leftover pad slots.
    KP = K * P
    glloc = np.zeros((n_cores, B * P), np.int64)
    for c in range(n_cores):
        for b in range(B):
            sl = slice(b * KP, (b + 1) * KP)
            srcv = src_sh[c, sl]
            is_real = amask_sh[c, sl] != 0
            is_loc = is_real & (srcv // NLp == c)
            loc_idx = np.where(is_loc)[0]
            loc_idx = loc_idx[np.argsort(srcv[loc_idx], kind="stable")]
            head, rest = loc_idx[:P], loc_idx[P:]
            rem_idx = np.where(is_real & ~is_loc)[0]
            mid = np.concatenate([rest, rem_idx])
            mid = mid[np.argsort(srcv[mid], kind="stable")]
            pad_idx = np.where(~is_real)[0]
            npad0 = P - len(head)
            order_b = np.concatenate(
                [head, pad_idx[:npad0], mid, pad_idx[npad0:]])
            assert len(order_b) == KP
            for arr in (src_sh, dstc_sh, amask_sh, eid_sh):
                arr[c, sl] = arr[c, sl][order_b]
            ea_sh[c, sl] = ea_sh[c, sl][order_b]
            c0 = src_sh[c, sl][:P].astype(np.int64) - c * NLp
            c0[~(amask_sh[c, sl][:P] != 0)] = 0
            assert ((c0 >= 0) & (c0 < NLp)).all()
            glloc[c, b * P:(b + 1) * P] = c0

    # host-precomputed one-hot scatter/gather matrices (both orientations)
    # and 5-wide edge attrs (4 attrs + pad-bias col routed through qwk).
    dstc_i = dstc_sh.astype(np.int32)                      # [cores, Ep]
    iota = np.arange(P, dtype=np.int32)
    Sblk_sh = np.zeros((n_cores, P, C, P), np.float16)
    St_sh = np.zeros((n_cores, P, C, P), np.float16)
    for c in range(n_cores):
        d = dstc_i[c].reshape(C, P)                        # [chunk, slotP]
        real = amask_sh[c].reshape(C, P) != 0
        oh = (d[:, :, None] == iota[None, None, :])        # [chunk, slotP, dst]
        oh &= real[:, :, None]
        Sblk_sh[c] = oh.transpose(1, 0, 2).astype(np.float16)   # [p=slot, chunk, dst]
        St_sh[c] = oh.transpose(2, 0, 1).astype(np.float16)     # [p=dst, chunk, slot]


    xsp = np.concatenate([np.asarray(x, np.float32),
                          np.asarray(splitter_probs, np.float32)], axis=1)
    xsp_p = np.zeros((Np, XS), np.float32)
    valid_new = old_of_new >= 0
    xsp_p[valid_new] = xsp[old_of_new[valid_new]]
    gids_p = np.zeros(Np, np.int32)
    gids_p[valid_new] = gids0[old_of_new[valid_new]].astype(np.int32)

    def wrap16(idx):
        """flat int array -> [128, n//16] int16 wrapped-16 + replicated."""
        n = idx.shape[0]
        assert n % 16 == 0
        arr = np.asarray(idx, np.int16).reshape(n // 16, 16).T  # [16, n//16]
        return np.ascontiguousarray(np.tile(arr, (8, 1)))       # [128, n//16]

    ZLO = 0        # a zero row in the low window
    ZHI = 8320     # zero rows at 41088.. in the hi window (base 32768)
    HIC = 3        # trailing chunks per block that may hold high-table rows
    shards = []
    for c in range(n_cores):
        lo = c * NLp
        src_g = src_sh[c].astype(np.int64) + 128
        assert ((src_g >= 32768).reshape(C, P)
                .reshape(B, K, P)[:, 1:K - HIC, :].sum() == 0), \
            "high-table edges below the trailing chunks; raise HIC"
        gilo = np.where(src_g < 32768, src_g, ZLO)
        gilo.reshape(B, K, P)[:, 0, :] = ZLO    # chunk 0 handled via glloc
        gihi_full = np.where(src_g >= 32768, src_g - 32768, ZHI)
        gihi = gihi_full.reshape(B, K, P)[:, K - HIC:, :].reshape(-1)
        shards.append(dict(
            gilo=wrap16(gilo),                                            # [128, Ep/16] i16
            gihi=wrap16(gihi),                                        # [128, B*HIC*8] i16
            glloc=wrap16(glloc[c]),                                       # [128, B*8] i16
            t12i=wrap16(gids_p[lo:lo + NLp]),                             # [128, NLp/16] i16
            Sblk=np.ascontiguousarray(
                Sblk_sh[c].reshape(P, B, K * P)),                         # [128, B, K*128] f16
            Stt=np.ascontiguousarray(
                St_sh[c].reshape(P, B, K * P)),                           # [128, B, K*128] f16
            eaT=np.ascontiguousarray(ea_sh[c].reshape(Ep, EA).T),         # [4, Ep] f16
            eaE=np.ascontiguousarray(
                ea_sh[c].reshape(C, P, EA).transpose(1, 0, 2)),           # [128, C, 4] f16
            xspT=np.ascontiguousarray(xsp_p[lo:lo + NLp].T),              # [7, NLp] f32
        ))

    has_pad = (eid_sh < 0).reshape(n_cores, C, P).any(axis=(0, 2))
    meta_src = src_sh.copy()
    meta = dict(N=N, E=E, Np=Np, NLp=NLp, B=B, K=K, C=C, Ep=Ep, eid_sh=eid_sh,
                chunk_pad=[bool(v) for v in has_pad], src_sh=meta_src)
    return shards, meta


def _pack_rows(W, dtype, nck=2):
    """[R, X] -> [128, nck, X] partition-chunked, zero padded."""
    W = np.asarray(W, dtype)
    R, X = W.shape
    out = np.zeros((P, nck, X), dtype)
    for ci in range(nck):
        r0 = ci * P
        rl = min(P, max(0, R - r0))
        if rl > 0:
            out[:rl, ci] = W[r0:r0 + rl]
    return out


def _host_weights(group_probs, endpoint_preds,
                  W_in, b_in, Wq, Wk, Wv, We, Wo, bo, ln1_g, ln1_b,
                  W_ff1, b_ff1, W_ff2, b_ff2, ln2_g, ln2_b,
                  W_e1, b_e1, W_e2, b_e2, W_e3, b_e3):
    def bc(v, X):
        return np.broadcast_to(np.asarray(v, f16)[None, :], (P, X)).copy()

    w = {}
    W_in = np.asarray(W_in, f32)
    Wxs8 = np.zeros((XS + 1, HID), f32)
    Wxs8[0:4] = W_in[0:4]
    Wxs8[4:7] = W_in[7:10]
    Wxs8[7] = np.asarray(b_in, f32)
    w["Wxs8"] = Wxs8
    w["Wgp"] = np.ascontiguousarray(W_in[4:7])
    w["Wep"] = np.ascontiguousarray(W_in[10:16])
    w["gpT"] = np.ascontiguousarray(np.asarray(group_probs, f32).T)
    w["epT"] = np.ascontiguousarray(np.asarray(endpoint_preds, f32).T)

    scale = f32(1.0 / np.sqrt(DH))
    for l in range(L):
        Wq_s = np.asarray(Wq[l], f32) * scale
        We_l = np.asarray(We[l], f32)           # [4, 192]
        # M_k[r, h*4+j] = sum_d Wq_s[r, h*48+d] * We[j, h*48+d]
        M_k = np.zeros((HID, HEADS * EA), f32)
        for h in range(HEADS):
            M_k[:, h * EA:(h + 1) * EA] = (
                Wq_s[:, h * DH:(h + 1) * DH] @ We_l[:, h * DH:(h + 1) * DH].T)
        w[f"WqM{l}"] = _pack_rows(np.concatenate([Wq_s, M_k], axis=1), f16)  # [128,2,208]
        w[f"WkWv{l}"] = _pack_rows(np.concatenate(
            [np.asarray(Wk[l], f32), np.asarray(Wv[l], f32)], axis=1), f16)  # [128,2,384]
        # block-diagonal We for the v-side moment matmul: [16, 192]
        Wblk = np.zeros((HEADS * EA, HID), f16)
        for h in range(HEADS):
            Wblk[h * EA:(h + 1) * EA, h * DH:(h + 1) * DH] = (
                We_l[:, h * DH:(h + 1) * DH].astype(f16))
        w[f"Wblk{l}"] = Wblk
        Wo_p = _pack_rows(Wo[l], f16)
        Wo_p[HID - P, 1] = np.asarray(bo[l], f16)     # ones-row bias fold
        w[f"Wo{l}"] = Wo_p
        w[f"ln1g{l}"] = bc(ln1_g[l], HID)
        w[f"ln1b{l}"] = bc(ln1_b[l], HID)
        Wff1_p = _pack_rows(W_ff1[l], f16)
        Wff1_p[HID - P, 1] = np.asarray(b_ff1[l], f16)
        w[f"Wff1_{l}"] = Wff1_p                       # [128,2,384]
        w[f"Wff2_{l}"] = _pack_rows(W_ff2[l], f16, nck=3)   # [128,3,192]
        w[f"bff2r{l}"] = np.asarray(b_ff2[l], f16)[None, :]  # [1,192]
        w[f"ln2g{l}"] = bc(ln2_g[l], HID)
        w[f"ln2b{l}"] = bc(ln2_b[l], HID)

    W_e1 = np.asarray(W_e1, f32)
    w["W1a"] = _pack_rows(W_e1[0:HID], f16)
    w["W1b"] = _pack_rows(W_e1[HID:2 * HID], f16)
    W1c5 = np.zeros((EA + 1, HID), f16)
    W1c5[0:EA] = np.asarray(W_e1[2 * HID:], f16)
    W1c5[EA] = np.asarray(b_e1, f16)
    w["W1c5"] = W1c5
    W2e = _pack_rows(W_e2, f16)                       # [128,2,96]
    W2e[HID - P, 1] = np.asarray(b_e2, f16)
    w["W2e"] = W2e
    W3e = np.zeros((Z2 + 1, 1), f16)
    W3e[0:Z2] = np.asarray(W_e3, f16)
    W3e[Z2, 0] = np.asarray(b_e3, f16).reshape(-1)[0]
    w["W3e"] = W3e
    return w


# ----------------------------------------------------------------------------
# device program
# ----------------------------------------------------------------------------

def build_program(meta, n_cores, debug=False):
    B, K, C, Ep, NLp, Np = (meta["B"], meta["K"], meta["C"], meta["Ep"],
                            meta["NLp"], meta["Np"])
    chunk_pad = meta.get("chunk_pad", [True] * C)
    GW = 8    # chunks per indirect gather
    W = 5     # chunks per compute batch
    LNW = 4   # blocks per layer-norm batch

    nc = bass.Bass()

    def param(name, shape, dtype):
        return nc.declare_dram_parameter(name, list(shape), dtype, isOutput=False)

    gilo_p = param("gilo", [P, Ep // 16], dt.int16)
    HIC = 3
    gihi_p = param("gihi", [P, B * HIC * 8], dt.int16)
    glloc_p = param("glloc", [P, B * 8], dt.int16)
    t12i_p = param("t12i", [P, NLp // 16], dt.int16)
    Sblk_p = param("Sblk", [P, B, K * P], dt.float16)
    Stt_p = param("Stt", [P, B, K * P], dt.float16)
    eaT = param("eaT", [EA, Ep], dt.float16)
    eaE_p = param("eaE", [P, C, EA], dt.float16)
    xspT = param("xspT", [XS, NLp], dt.float32)
    Wxs8 = param("Wxs8", [XS + 1, HID], dt.float32)
    Wgp = param("Wgp", [GP, HID], dt.float32)
    Wep = param("Wep", [EP, HID], dt.float32)
    gpT = param("gpT", [GP, G_TBL], dt.float32)
    epT = param("epT", [EP, G_TBL], dt.float32)
    WqM_p = [param(f"WqM{l}", [P, 2, QW], dt.float16) for l in range(L)]
    WkWv_p = [param(f"WkWv{l}", [P, 2, H2], dt.float16) for l in range(L)]
    Wblk_p = [param(f"Wblk{l}", [HEADS * EA, HID], dt.float16) for l in range(L)]
    Wo_p = [param(f"Wo{l}", [P, 2, HID], dt.float16) for l in range(L)]
    ln1g_p = [param(f"ln1g{l}", [P, HID], dt.float16) for l in range(L)]
    ln1b_p = [param(f"ln1b{l}", [P, HID], dt.float16) for l in range(L)]
    Wff1_p = [param(f"Wff1_{l}", [P, 2, FFN], dt.float16) for l in range(L)]
    Wff2_p = [param(f"Wff2_{l}", [P, 3, HID], dt.float16) for l in range(L)]
    bff2r_p = [param(f"bff2r{l}", [1, HID], dt.float16) for l in range(L)]
    ln2g_p = [param(f"ln2g{l}", [P, HID], dt.float16) for l in range(L)]
    ln2b_p = [param(f"ln2b{l}", [P, HID], dt.float16) for l in range(L)]
    W1a = param("W1a", [P, 2, HID], dt.float16)
    W1b = param("W1b", [P, 2, HID], dt.float16)
    W1c5 = param("W1c5", [EA + 1, HID], dt.float16)
    W2e = param("W2e", [P, 2, Z2], dt.float16)
    W3e = param("W3e", [Z2 + 1, 1], dt.float16)

    out_z = nc.declare_dram_parameter("out_z", [1, Ep], dt.float32, isOutput=True)
    if debug:
        dbg_h0 = nc.declare_dram_parameter("dbg_h0", [P, B, HID], dt.float32, isOutput=True)
        dbg_q = nc.declare_dram_parameter("dbg_q", [P, B, QW], dt.float32, isOutput=True)
        dbg_kvt = nc.declare_dram_parameter("dbg_kvt", [Np, H2], dt.float16, isOutput=True)
        dbg_msg = nc.declare_dram_parameter("dbg_msg", [P, B, HID], dt.float32, isOutput=True)
        dbg_h1 = nc.declare_dram_parameter("dbg_h1", [P, B, HID], dt.float32, isOutput=True)
        dbg_h3 = nc.declare_dram_parameter("dbg_h3", [P, B, HID], dt.float32, isOutput=True)
        dbg_u = nc.declare_dram_parameter("dbg_u", [Np, HID], dt.float16, isOutput=True)
        dbg_w = nc.declare_dram_parameter("dbg_w", [P, B, HID], dt.float32, isOutput=True)

    def dbg_dump_h(pool, nc_, dst, src_tile):
        for b_ in range(B):
            t_ = pool.tile([P, HID], dt.float32, tag="dbgc", bufs=1)
            nc_.vector.tensor_copy(out=t_[:], in_=src_tile[:, b_, :])
            nc_.sync.dma_start(out=dst[:, b_, :], in_=t_[:])

    NpA = Np + 256           # 128 zero rows before and after the data
    UW = 256                 # u rows padded to 256 f16 (512B, dma_gather elem)
    kv_loc = nc.dram_tensor("kv_loc", [NLp, H2], dt.float16)
    kv_tbl = nc.dram_tensor("kv_tbl", [NpA, H2], dt.float16, addr_space="Shared")
    u_loc = nc.dram_tensor("u_loc", [NLp, UW], dt.float16)
    u_tbl = nc.dram_tensor("u_tbl", [NpA, UW], dt.float16, addr_space="Shared")
    t12_d = nc.dram_tensor("t12_d", [G_TBL, UW], dt.float16)

    from contextlib import ExitStack
    with nc.allow_low_precision("f16 reduces/logits are within the 2e-2 budget"), \
         tile.TileContext(nc) as tc:
        with tc.tile_pool(name="pers", bufs=1) as pers, \
             tc.tile_pool(name="wp", bufs=1) as wpool:

            # ---------------- persistent state ----------------
            h_loc = pers.tile([P, B, HID], dt.float16)
            hT0 = pers.tile([P, NLp], dt.float16)
            hT1 = pers.tile([HID - P + 1, NLp], dt.float16)   # row 64 = ones
            q_loc = pers.tile([P, B, QW], dt.float16)
            msg_loc = pers.tile([P, B, HID], dt.float16)
            accL = pers.tile([P, B, ACC], dt.float16)

            ident32 = pers.tile([P, P], dt.float32)
            make_identity(nc, ident32[:])
            ident16 = pers.tile([P, P], dt.float16)
            nc.vector.tensor_copy(out=ident16[:], in_=ident32[:])
            ones1 = pers.tile([1, P], dt.float16)
            nc.vector.memset(ones1[:], 1.0)
            nc.vector.memset(hT1[:], 1.0)

            eaE_t = pers.tile([P, C, EA], dt.float16)
            nc.sync.dma_start(out=eaE_t[:], in_=eaE_p[:, :, :])
            from concourse import library_config
            nc.gpsimd.load_library(library_config.mlp)
            regN = {}
            for n_ in range(P, 1024 + P, P):
                regN[n_] = nc.gpsimd.to_reg(n_)
            # zero-row aprons for the gather tables
            zkv = pers.tile([P, H2], dt.float16)  # (kept small)
            nc.vector.memset(zkv[:], 0.0)
            nc.sync.dma_start(out=kv_tbl[0:P, :], in_=zkv[:])
            nc.sync.dma_start(out=kv_tbl[P + Np:NpA, :], in_=zkv[:])
            zu = pers.tile([P, UW], dt.float16)
            nc.vector.memset(zu[:], 0.0)
            nc.sync.dma_start(out=u_tbl[0:P, :], in_=zu[:])
            nc.sync.dma_start(out=u_tbl[P + Np:NpA, :], in_=zu[:])

            def wtile(pp, shape, dtype, tag):
                t_ = wpool.tile(list(shape), dtype, tag=tag)
                nc.sync.dma_start(out=t_[:], in_=pp[...])
                return t_

            _setup_stack = ExitStack()
            sup = _setup_stack.enter_context(tc.tile_pool(name="setup", bufs=2))
            psU = _setup_stack.enter_context(
                tc.tile_pool(name="psU", bufs=2, space="PSUM"))

            # ---------------- T12 [512, 192] -> DRAM ----------------
            gpT_t = wtile(gpT, [GP, G_TBL], dt.float32, "gpT")
            epT_t = wtile(epT, [EP, G_TBL], dt.float32, "epT")
            Wgp_t = wtile(Wgp, [GP, HID], dt.float32, "Wgp")
            Wep_t = wtile(Wep, [EP, HID], dt.float32, "Wep")
            T12s = sup.tile([P, G_TBL // P, HID], dt.float16, tag="T12", bufs=1)
            for gc in range(G_TBL // P):
                pt = psU.tile([P, HID], dt.float32, tag="t12")
                nc.tensor.matmul(out=pt[:], lhsT=gpT_t[:, gc * P:(gc + 1) * P],
                                 rhs=Wgp_t[:], start=True, stop=False)
                nc.tensor.matmul(out=pt[:], lhsT=epT_t[:, gc * P:(gc + 1) * P],
                                 rhs=Wep_t[:], start=False, stop=True)
                nc.scalar.copy(out=T12s[:, gc, :], in_=pt[:])
            nc.sync.dma_start(
                out=t12_d[:, 0:HID].rearrange("(gc p) f -> p gc f", p=P),
                in_=T12s[:])

            # ---------------- h0 ----------------
            xspT_t = sup.tile([XS + 1, NLp], dt.float32, tag="xsp", bufs=1)
            nc.vector.memset(xspT_t[:], 1.0)
            nc.sync.dma_start(out=xspT_t[0:XS, :], in_=xspT[:, :])
            Wxs_t = wtile(Wxs8, [XS + 1, HID], dt.float32, "Wxs")
            t12i_t = sup.tile([P, NLp // 16], dt.int16, tag="t12i", bufs=1)
            nc.sync.dma_start(out=t12i_t[:], in_=t12i_p[:, :])
            t12g = sup.tile([P, B, UW], dt.float16, tag="t12g", bufs=1)
            for g_ in range(0, NLp, 1024):
                gn_ = min(1024, NLp - g_)
                nc.gpsimd.dma_gather(t12g[:, g_ // P:(g_ + gn_) // P, :],
                                     t12_d[:, :],
                                     t12i_t[:, g_ // 16:(g_ + gn_) // 16],
                                     gn_, regN[gn_], UW)
            for b in range(B):
                hp = psU.tile([P, HID], dt.float32, tag="h0")
                nc.tensor.matmul(out=hp[:], lhsT=xspT_t[:, b * P:(b + 1) * P],
                                 rhs=Wxs_t[:], start=True, stop=True)
                nc.vector.tensor_tensor(out=h_loc[:, b, :], in0=hp[:],
                                        in1=t12g[:, b, 0:HID],
                                        op=mybir.AluOpType.add)
            if debug:
                dbg_dump_h(sup, nc, dbg_h0, h_loc)
            _setup_stack.close()

            _work = ExitStack()
            sb = _work.enter_context(tc.tile_pool(name="sbN", bufs=3))
            sbE = _work.enter_context(tc.tile_pool(name="sbE", bufs=3))
            gat = _work.enter_context(tc.tile_pool(name="gat", bufs=3))
            sbS = _work.enter_context(tc.tile_pool(name="sbS", bufs=2))
            _tp = ExitStack()
            psT = _tp.enter_context(tc.tile_pool(name="psT", bufs=2, space="PSUM"))

            # ---------------- helpers ----------------
            def transpose_h(b):
                """h_loc[:, b, :] (f16) -> hT0/hT1 columns b*P:(b+1)*P."""
                for ci, (f0, fl) in enumerate(((0, P), (P, HID - P))):
                    tp = psT.tile([P, P], dt.float16, tag="tp16")
                    nc.tensor.transpose(out=tp[:fl, :], in_=h_loc[:, b, f0:f0 + fl],
                                        identity=ident16[:])
                    dstt = hT0 if ci == 0 else hT1
                    nc.scalar.copy(out=dstt[:fl, b * P:(b + 1) * P], in_=tp[:fl, :])

            def layer_norm_batch(b0, bn, g_t, b_t):
                """LN over h_loc[:, b0:b0+bn, :] in place (h is f16).
                var = E[h^2] - m^2 (f32 accumulators); normalize via one
                fused (h - m) * rs tensor_scalar per block (4x mode)."""
                hv = h_loc[:, b0:b0 + bn, :]
                red = sb.tile([P, LNW], dt.float32, tag="ln_m")
                nc.vector.tensor_reduce(out=red[:, 0:bn], in_=hv,
                                        axis=mybir.AxisListType.X,
                                        op=mybir.AluOpType.add)
                m = sb.tile([P, LNW], dt.float32, tag="ln_mm")
                nc.vector.tensor_scalar_mul(out=m[:, 0:bn], in0=red[:, 0:bn],
                                            scalar1=1.0 / HID)
                sq = sb.tile([P, LNW, HID], dt.float16, tag="ln_sq", bufs=2)
                nc.scalar.square(out=sq[:, 0:bn, :], in_=hv)
                v = sb.tile([P, LNW], dt.float32, tag="ln_v")
                nc.vector.tensor_reduce(out=v[:, 0:bn], in_=sq[:, 0:bn, :],
                                        axis=mybir.AxisListType.X,
                                        op=mybir.AluOpType.add)
                m2 = sb.tile([P, LNW], dt.float32, tag="ln_m2")
                nc.vector.tensor_tensor(out=m2[:, 0:bn], in0=m[:, 0:bn],
                                        in1=m[:, 0:bn],
                                        op=mybir.AluOpType.mult)
                ve = sb.tile([P, LNW], dt.float32, tag="ln_ve")
                nc.vector.tensor_scalar(out=ve[:, 0:bn], in0=v[:, 0:bn],
                                        scalar1=1.0 / HID, scalar2=1e-5,
                                        op0=mybir.AluOpType.mult,
                                        op1=mybir.AluOpType.add)
                nc.vector.tensor_tensor(out=ve[:, 0:bn], in0=ve[:, 0:bn],
                                        in1=m2[:, 0:bn],
                                        op=mybir.AluOpType.subtract)
                rv = sb.tile([P, LNW], dt.float32, tag="ln_rv")
                nc.vector.reciprocal(out=rv[:, 0:bn], in_=ve[:, 0:bn])
                rs = sb.tile([P, LNW], dt.float32, tag="ln_rs")
                nc.scalar.sqrt(out=rs[:, 0:bn], in_=rv[:, 0:bn])
                xc = sb.tile([P, LNW, HID], dt.float16, tag="ln_xc", bufs=2)
                for j in range(bn):
                    nc.vector.tensor_scalar(out=xc[:, j, :],
                                            in0=h_loc[:, b0 + j, :],
                                            scalar1=m[:, j:j + 1],
                                            scalar2=rs[:, j:j + 1],
                                            op0=mybir.AluOpType.subtract,
                                            op1=mybir.AluOpType.mult)
                nc.vector.tensor_tensor(out=xc[:, 0:bn, :], in0=xc[:, 0:bn, :],
                                        in1=g_t[:].rearrange("p (o f) -> p o f", o=1)
                                            .to_broadcast([P, bn, HID]),
                                        op=mybir.AluOpType.mult)
                nc.vector.tensor_tensor(out=hv, in0=xc[:, 0:bn, :],
                                        in1=b_t[:].rearrange("p (o f) -> p o f", o=1)
                                            .to_broadcast([P, bn, HID]),
                                        op=mybir.AluOpType.add)

            # ---------------- layers ----------------
            _gatE = ExitStack()
            gatE = _gatE.enter_context(tc.tile_pool(name="gatE", bufs=2))
            for l in range(L):
                WqM_t = wtile(WqM_p[l], [P, 2, QW], dt.float16, "WqM")
                WkWv_t = wtile(WkWv_p[l], [P, 2, H2], dt.float16, "WkWv")
                Wblk_t = wtile(Wblk_p[l], [HEADS * EA, HID], dt.float16, "Wblk")
                Wo_t = wtile(Wo_p[l], [P, 2, HID], dt.float16, "Wo")
                ln1g_t = wtile(ln1g_p[l], [P, HID], dt.float16, "ln1g")
                ln1b_t = wtile(ln1b_p[l], [P, HID], dt.float16, "ln1b")
                Wff1_t = wtile(Wff1_p[l], [P, 2, FFN], dt.float16, "Wff1")
                Wff2_t = wtile(Wff2_p[l], [P, 3, HID], dt.float16, "Wff2")
                bff2r_t = wtile(bff2r_p[l], [1, HID], dt.float16, "bff2r")
                ln2g_t = wtile(ln2g_p[l], [P, HID], dt.float16, "ln2g")
                ln2b_t = wtile(ln2b_p[l], [P, HID], dt.float16, "ln2b")

                # --- qkv phase ---
                _ph = ExitStack()
                psK = _ph.enter_context(
                    tc.tile_pool(name="psK", bufs=2, space="PSUM"))
                for b in range(B):
                    transpose_h(b)
                for b in range(B):
                    qp = psK.tile([P, QW], dt.float32, tag="qp")
                    kvp = psK.tile([P, H2], dt.float32, tag="kvp")
                    for ci, (f0, fl) in enumerate(((0, P), (P, HID - P))):
                        hTt = hT0 if ci == 0 else hT1
                        lhs = hTt[:fl, b * P:(b + 1) * P]
                        nc.tensor.matmul(out=qp[:], lhsT=lhs, rhs=WqM_t[:fl, ci, :],
                                         start=(ci == 0), stop=(ci == 1))
                        nc.tensor.matmul(out=kvp[:], lhsT=lhs, rhs=WkWv_t[:fl, ci, :],
                                         start=(ci == 0), stop=(ci == 1))
                    nc.vector.tensor_copy(out=q_loc[:, b, :], in_=qp[:])
                    kvf = sb.tile([P, H2], dt.float16, tag="kvf")
                    nc.vector.tensor_copy(out=kvf[:], in_=kvp[:])
                    nc.sync.dma_start(out=kv_loc[b * P:(b + 1) * P, :], in_=kvf[:])
                _ph.close()

                nc.gpsimd.collective_compute(
                    "AllGather", mybir.AluOpType.bypass,
                    replica_groups=[list(range(n_cores))],
                    ins=[kv_loc[:, :]], outs=[kv_tbl[P:P + Np, :]])
                if debug and l == 0:
                    for b_ in range(B):
                        tq = sb.tile([P, QW], dt.float32, tag="dbgq", bufs=1)
                        nc.vector.tensor_copy(out=tq[:], in_=q_loc[:, b_, :])
                        nc.sync.dma_start(out=dbg_q[:, b_, :], in_=tq[:])
                    for bb_ in range(Np // P):
                        tk = sb.tile([P, H2], dt.float16, tag="dbgk", bufs=1)
                        nc.sync.dma_start(out=tk[:],
                                          in_=kv_tbl[P + bb_ * P:P + (bb_ + 1) * P, :])
                        nc.sync.dma_start(out=dbg_kvt[bb_ * P:(bb_ + 1) * P, :], in_=tk[:])

                # --- edge phase ---
                _ph = ExitStack()
                psQ = _ph.enter_context(
                    tc.tile_pool(name="psQd", bufs=2, space="PSUM"))
                ps = _ph.enter_context(
                    tc.tile_pool(name="psSt", bufs=2, space="PSUM"))
                psAcc = _ph.enter_context(
                    tc.tile_pool(name="psAcc", bufs=2, space="PSUM"))

                def edge_X(qd16, kv4, t0, wn):
                    """per-edge logits -> X = [a | a*ea | a*v] for wn chunks.
                    q.k and qwk.ea partial products land in one [h, 52]
                    layout so a single reduce yields the logit."""
                    prc = sbE.tile([P, W, HEADS, DH + EA], dt.float16,
                                   tag="pr", bufs=2)
                    nc.vector.tensor_tensor(
                        out=prc[:, 0:wn, :, 0:DH],
                        in0=qd16[:, 0:wn, 0:HID].rearrange(
                            "p w (h d) -> p w h d", h=HEADS),
                        in1=kv4[:, :, 0:HID].rearrange(
                            "p w (h d) -> p w h d", h=HEADS),
                        op=mybir.AluOpType.mult)
                    nc.vector.tensor_tensor(
                        out=prc[:, 0:wn, :, DH:DH + EA],
                        in0=qd16[:, 0:wn, HID:QW].rearrange(
                            "p w (h j) -> p w h j", h=HEADS),
                        in1=eaE_t[:, t0:t0 + wn, :].rearrange(
                            "p w (o j) -> p w o j", o=1)
                            .to_broadcast([P, wn, HEADS, EA]),
                        op=mybir.AluOpType.mult)
                    lg = sbE.tile([P, W, HEADS], dt.float16, tag="lg")
                    nc.vector.tensor_reduce(
                        out=lg[:, 0:wn, :],
                        in_=prc[:, 0:wn, :, :].rearrange(
                            "p w h x -> p (w h) x"),
                        axis=mybir.AxisListType.X, op=mybir.AluOpType.add)
                    X = sbE.tile([P, W, ACC], dt.float16, tag="X", bufs=2)
                    nc.scalar.activation(out=X[:, 0:wn, 0:HEADS],
                                         in_=lg[:, 0:wn, :],
                                         func=mybir.ActivationFunctionType.Exp)
                    nc.vector.tensor_tensor(
                        out=X[:, 0:wn, HEADS:HEADS + HEADS * EA].rearrange(
                            "p w (h j) -> p w h j", h=HEADS),
                        in0=X[:, 0:wn, 0:HEADS].rearrange(
                            "p w (h o) -> p w h o", o=1)
                            .to_broadcast([P, wn, HEADS, EA]),
                        in1=eaE_t[:, t0:t0 + wn, :].rearrange(
                            "p w (o j) -> p w o j", o=1)
                            .to_broadcast([P, wn, HEADS, EA]),
                        op=mybir.AluOpType.mult)
                    nc.vector.tensor_tensor(
                        out=X[:, 0:wn, HEADS + HEADS * EA:].rearrange(
                            "p w (h d) -> p w h d", h=HEADS),
                        in0=X[:, 0:wn, 0:HEADS].rearrange(
                            "p w (h o) -> p w h o", o=1)
                            .to_broadcast([P, wn, HEADS, DH]),
                        in1=kv4[:, :, HID:].rearrange(
                            "p w (h d) -> p w h d", h=HEADS),
                        op=mybir.AluOpType.mult)
                    return X

                # pass L: each block's chunk 0 (local-src edges) gathers from
                # kv_loc and runs while the AllGather is still in flight;
                # partial acc saved to accL.  tile_wait_until makes the Tile
                # scheduler order all pass-L work before the collective-gated
                # pass-R gathers (else a pass-R gather head-of-line blocks the
                # Pool queue for the whole transfer).
                _vt = ExitStack()
                _vt.enter_context(tc.tile_wait_until(100 + 20 * l))
                for b in range(B):
                    S0 = sbS.tile([P, P], dt.float16, tag="S0")
                    nc.sync.dma_start(out=S0[:], in_=Sblk_p[:, b, 0:P])
                    St0 = sbS.tile([P, P], dt.float16, tag="St0")
                    nc.sync.dma_start(out=St0[:], in_=Stt_p[:, b, 0:P])
                    lidx = gatE.tile([P, 8], dt.int16, tag="lidx", bufs=2)
                    nc.sync.dma_start(out=lidx[:],
                                      in_=glloc_p[:, b * 8:(b + 1) * 8])
                    kv0 = gatE.tile([P, 1, H2], dt.float16, tag="kv0", bufs=2)
                    nc.gpsimd.dma_gather(kv0[:, :, :], kv_loc[:, :],
                                         lidx[:, :], P, regN[P], H2)
                    qdp = psQ.tile([P, 2, 256], dt.float32, tag="qdp")
                    nc.tensor.matmul(out=qdp[:, 0, 0:QW], lhsT=St0[:],
                                     rhs=q_loc[:, b, :], start=True, stop=True)
                    qd16 = sbE.tile([P, W, QW], dt.float16, tag="qd16", bufs=2)
                    nc.scalar.copy(out=qd16[:, 0:1, :], in_=qdp[:, 0:1, 0:QW])
                    X = edge_X(qd16, kv0[:, 0:1, :], b * K, 1)
                    acc0 = psAcc.tile([P, ACC], dt.float32, tag="acc")
                    nc.tensor.matmul(out=acc0[:], lhsT=S0[:], rhs=X[:, 0, :],
                                     start=True, stop=True,
                                     skip_group_check=True)
                    nc.scalar.copy(out=accL[:, b, :], in_=acc0[:])

                _vt.close()
                # pass R: remote chunks 1..K-1, after the AllGather lands.
                _vt = ExitStack()
                _vt.enter_context(tc.tile_wait_until(110 + 20 * l))
                for b in range(B):
                    S_t = sbS.tile([P, K, P], dt.float16, tag="Sblk")
                    nc.sync.dma_start(
                        out=S_t[:].rearrange("p k j -> p (k j)"),
                        in_=Sblk_p[:, b, :])
                    St_t = sbS.tile([P, K, P], dt.float16, tag="Stt")
                    nc.sync.dma_start(
                        out=St_t[:].rearrange("p k j -> p (k j)"),
                        in_=Stt_p[:, b, :])
                    acc = psAcc.tile([P, ACC], dt.float32, tag="acc")
                    nc.tensor.matmul(out=acc[:], lhsT=ident16[:],
                                     rhs=accL[:, b, :], start=True, stop=False,
                                     skip_group_check=True)
                    S16 = K * P // 16
                    SH16 = HIC * P // 16
                    ilo = gatE.tile([P, S16], dt.int16, tag="ilo", bufs=2)
                    nc.sync.dma_start(out=ilo[:], in_=gilo_p[:, b * S16:(b + 1) * S16])
                    ihi = gatE.tile([P, SH16], dt.int16, tag="ihi", bufs=2)
                    nc.sync.dma_start(out=ihi[:],
                                      in_=gihi_p[:, b * SH16:(b + 1) * SH16])
                    kvL = gatE.tile([P, K, H2], dt.float16, tag="kvL", bufs=2)
                    kvH = gatE.tile([P, HIC, H2], dt.float16, tag="kvH", bufs=2)
                    c0 = 1
                    while c0 < K:
                        cn = min(8, K - c0)
                        nc.gpsimd.dma_gather(
                            kvL[:, c0:c0 + cn, :], kv_tbl[0:32768, :],
                            ilo[:, c0 * 8:(c0 + cn) * 8],
                            cn * P, regN[cn * P], H2)
                        c0 += cn
                    nc.gpsimd.dma_gather(
                        kvH[:], kv_tbl[32768:NpA, :], ihi[:],
                        HIC * P, regN[HIC * P], H2)
                    for kk0 in range(1, K, W):
                        wn = min(W, K - kk0)
                        t0 = b * K + kk0
                        if kk0 + wn <= K - HIC:
                            kv4 = kvL[:, kk0:kk0 + wn, :]
                        else:
                            kvg = sbE.tile([P, W, H2], dt.float16, tag="kvg",
                                           bufs=2)
                            h0_ = max(kk0, K - HIC)
                            if h0_ > kk0:
                                nc.vector.tensor_copy(
                                    out=kvg[:, 0:h0_ - kk0, :],
                                    in_=kvL[:, kk0:h0_, :])
                            nc.vector.tensor_tensor(
                                out=kvg[:, h0_ - kk0:wn, :],
                                in0=kvL[:, h0_:kk0 + wn, :],
                                in1=kvH[:, h0_ - (K - HIC):kk0 + wn - (K - HIC), :],
                                op=mybir.AluOpType.add)
                            kv4 = kvg[:, 0:wn, :]
                        # qd: two W=2 PSUM tiles (bank-aligned slots of 256 f32)
                        qd16 = sbE.tile([P, W, QW], dt.float16, tag="qd16", bufs=2)
                        for half in range(0, wn, 2):
                            hn = min(2, wn - half)
                            qdp = psQ.tile([P, 2, 256], dt.float32, tag="qdp")
                            for j in range(hn):
                                nc.tensor.matmul(out=qdp[:, j, 0:QW],
                                                 lhsT=St_t[:, kk0 + half + j, :],
                                                 rhs=q_loc[:, b, :],
                                                 start=True, stop=True)
                            nc.scalar.copy(out=qd16[:, half:half + hn, :],
                                           in_=qdp[:, 0:hn, 0:QW])
                        X = edge_X(qd16, kv4, t0, wn)
                        for w_ in range(wn):
                            nc.tensor.matmul(out=acc[:], lhsT=S_t[:, kk0 + w_, :],
                                             rhs=X[:, w_, :],
                                             start=False, stop=False,
                                             skip_group_check=True)
                    # block tail: msg = (msgv + T @ Wblk) / den
                    T16 = sbE.tile([P, HEADS * EA], dt.float16, tag="T16")
                    nc.vector.tensor_copy(out=T16[:],
                                          in_=acc[:, HEADS:HEADS + HEADS * EA])
                    Ttp = ps.tile([HEADS * EA, P], dt.float32, tag="mm")
                    nc.tensor.matmul(out=Ttp[:], lhsT=T16[:], rhs=ident16[:],
                                     start=True, stop=True, skip_group_check=True)
                    Tt16 = sbE.tile([HEADS * EA, P], dt.float16, tag="Tt16")
                    nc.scalar.copy(out=Tt16[:], in_=Ttp[:])
                    nc.tensor.matmul(out=acc[:, HEADS + HEADS * EA:],
                                     lhsT=Tt16[:], rhs=Wblk_t[:],
                                     start=False, stop=True, skip_group_check=True)
                    den = sbE.tile([P, HEADS], dt.float32, tag="den")
                    nc.vector.tensor_scalar_add(out=den[:], in0=acc[:, 0:HEADS],
                                                scalar1=1e-16)
                    rden = sbE.tile([P, HEADS, 1], dt.float32, tag="rden")
                    nc.vector.reciprocal(
                        out=rden[:], in_=den[:].rearrange("p (h o) -> p h o", o=1))
                    nc.vector.tensor_tensor(
                        out=msg_loc[:, b, :].rearrange("p (h d) -> p h d", h=HEADS),
                        in0=acc[:, HEADS + HEADS * EA:].rearrange(
                            "p (h d) -> p h d", h=HEADS),
                        in1=rden[:].to_broadcast([P, HEADS, DH]),
                        op=mybir.AluOpType.mult)
                _vt.close()
                _ph.close()
                if debug and l == 0:
                    dbg_dump_h(sb, nc, dbg_msg, msg_loc)

                # --- node update: h = LN1(h + msg@Wo + bo) ---
                _ph = ExitStack()
                psN = _ph.enter_context(
                    tc.tile_pool(name="psN", bufs=2, space="PSUM"))
                for b0 in range(0, B, LNW):
                    bn = min(LNW, B - b0)
                    for b in range(b0, b0 + bn):
                        mT0 = sb.tile([P, P], dt.float16, tag="mT0")
                        mT1 = sb.tile([HID - P + 1, P], dt.float16, tag="mT1")
                        nc.vector.memset(mT1[:], 1.0)
                        for ci, (f0, fl) in enumerate(((0, P), (P, HID - P))):
                            tp = psT.tile([P, P], dt.float16, tag="tp16")
                            nc.tensor.transpose(out=tp[:fl, :],
                                                in_=msg_loc[:, b, f0:f0 + fl],
                                                identity=ident16[:])
                            nc.scalar.copy(out=(mT0 if ci == 0 else mT1)[:fl, :],
                                           in_=tp[:fl, :])
                        yp = psN.tile([P, HID], dt.float32, tag="yf")
                        nc.tensor.matmul(out=yp[:], lhsT=mT0[:], rhs=Wo_t[:, 0, :],
                                         start=True, stop=False)
                        nc.tensor.matmul(out=yp[:], lhsT=mT1[:],
                                         rhs=Wo_t[0:HID - P + 1, 1, :],
                                         start=False, stop=True)
                        nc.vector.tensor_tensor(out=h_loc[:, b, :],
                                                in0=h_loc[:, b, :], in1=yp[:],
                                                op=mybir.AluOpType.add)
                    layer_norm_batch(b0, bn, ln1g_t, ln1b_t)

                # --- FFN ---
                for b0 in range(0, B, LNW):
                    bn = min(LNW, B - b0)
                    for b in range(b0, b0 + bn):
                        transpose_h(b)
                        f1p = psN.tile([P, 3, P], dt.float32, tag="f1p")
                        for fc in range(3):
                            for ci, (f0, fl) in enumerate(((0, P), (P, HID - P + 1))):
                                hTt = hT0 if ci == 0 else hT1
                                nc.tensor.matmul(
                                    out=f1p[:, fc, :],
                                    lhsT=Wff1_t[:fl, ci, fc * P:(fc + 1) * P],
                                    rhs=hTt[:fl, b * P:(b + 1) * P],
                                    start=(ci == 0), stop=(ci == 1))
                        f1rT = sb.tile([P, 3, P], dt.float16, tag="f1rT", bufs=2)
                        nc.vector.tensor_scalar_max(out=f1rT[:], in0=f1p[:],
                                                    scalar1=0.0)
                        f2p = psN.tile([P, HID], dt.float32, tag="yf")
                        for fc in range(3):
                            nc.tensor.matmul(out=f2p[:], lhsT=f1rT[:, fc, :],
                                             rhs=Wff2_t[:, fc, :],
                                             start=(fc == 0), stop=False)
                        nc.tensor.matmul(out=f2p[:], lhsT=ones1[:], rhs=bff2r_t[:],
                                         start=False, stop=True)
                        nc.vector.tensor_tensor(out=h_loc[:, b, :],
                                                in0=h_loc[:, b, :], in1=f2p[:],
                                                op=mybir.AluOpType.add)
                    layer_norm_batch(b0, bn, ln2g_t, ln2b_t)
                _ph.close()
                if debug and l == 0:
                    dbg_dump_h(sb, nc, dbg_h1, h_loc)
                if debug and l == L - 1:
                    dbg_dump_h(sb, nc, dbg_h3, h_loc)

            _gatE.close()

            # ---------------- edge head ----------------
            W1a_t = wtile(W1a, [P, 2, HID], dt.float16, "W1a")
            W1b_t = wtile(W1b, [P, 2, HID], dt.float16, "W1b")
            W1c5_t = wtile(W1c5, [EA + 1, HID], dt.float16, "W1c5")
            W2e_t = wtile(W2e, [P, 2, Z2], dt.float16, "W2e")
            W3e_t = wtile(W3e, [Z2 + 1, 1], dt.float16, "W3e")
            w_loc = msg_loc   # reuse: msg dead after layer 2's node phase

            HW_ = 4    # chunks per edge-head compute batch
            _uw = ExitStack()
            psUW = _uw.enter_context(
                tc.tile_pool(name="psUW", bufs=2, space="PSUM"))

            for b in range(B):
                transpose_h(b)
            for b in range(B):
                up = psUW.tile([P, HID], dt.float32, tag="uw")
                wp_ = psUW.tile([P, HID], dt.float32, tag="uw")
                for ci, (f0, fl) in enumerate(((0, P), (P, HID - P))):
                    hTt = hT0 if ci == 0 else hT1
                    lhs = hTt[:fl, b * P:(b + 1) * P]
                    nc.tensor.matmul(out=up[:], lhsT=lhs, rhs=W1a_t[:fl, ci, :],
                                     start=(ci == 0), stop=(ci == 1))
                    nc.tensor.matmul(out=wp_[:], lhsT=lhs, rhs=W1b_t[:fl, ci, :],
                                     start=(ci == 0), stop=(ci == 1))
                uf16 = sb.tile([P, UW], dt.float16, tag="uf16")
                nc.scalar.copy(out=uf16[:, 0:HID], in_=up[:])
                nc.sync.dma_start(out=u_loc[b * P:(b + 1) * P, :], in_=uf16[:])
                nc.scalar.copy(out=w_loc[:, b, :], in_=wp_[:])

            if debug:
                for b_ in range(B):
                    tw = sb.tile([P, HID], dt.float32, tag="dbgc", bufs=1)
                    nc.vector.tensor_copy(out=tw[:], in_=w_loc[:, b_, :])
                    nc.sync.dma_start(out=dbg_w[:, b_, :], in_=tw[:])
            _uw.close()
            _tp.close()

            nc.gpsimd.collective_compute(
                "AllGather", mybir.AluOpType.bypass,
                replica_groups=[list(range(n_cores))],
                ins=[u_loc[:, :]], outs=[u_tbl[P:P + Np, :]])

            if debug:
                for bb_ in range(Np // P):
                    tu = sb.tile([P, UW], dt.float16, tag="dbgu", bufs=1)
                    nc.sync.dma_start(out=tu[:],
                                      in_=u_tbl[P + bb_ * P:P + (bb_ + 1) * P, :])
                    nc.sync.dma_start(out=dbg_u[bb_ * P:(bb_ + 1) * P, :],
                                      in_=tu[:, 0:HID])
            _head = ExitStack()
            gatU = _head.enter_context(tc.tile_pool(name="gatU", bufs=2))
            psH = _head.enter_context(
                tc.tile_pool(name="psH", bufs=2, space="PSUM"))
            psH2 = _head.enter_context(
                tc.tile_pool(name="psH2", bufs=3, space="PSUM"))

            # persistent ones-row pair tiles for the head
            ea5 = [pers.tile([EA + 1, K * P], dt.float16, name=f"ea5_{i}")
                   for i in range(2)]
            z1rT1 = [pers.tile([HID - P + 1, HW_ * P], dt.float16, name=f"z1rT1_{i}")
                     for i in range(3)]
            z2r97 = [pers.tile([Z2 + 1, HW_ * P], dt.float16, name=f"z2r97_{i}")
                     for i in range(3)]
            for i in range(2):
                nc.vector.memset(ea5[i][:], 1.0)
            for i in range(3):
                nc.vector.memset(z1rT1[i][:], 1.0)
                nc.vector.memset(z2r97[i][:], 1.0)

            def head_tail(z1T, cw, t0, bi):
                z1rT0 = sbE.tile([P, HW_ * P], dt.float16, tag="z1rT0", bufs=2)
                nc.vector.tensor_scalar_max(out=z1rT0[:, 0:cw],
                                            in0=z1T[:, 0, 0:cw], scalar1=0.0)
                z1b = z1rT1[bi % 3]
                nc.vector.tensor_scalar_max(out=z1b[0:HID - P, 0:cw],
                                            in0=z1T[0:HID - P, 1, 0:cw],
                                            scalar1=0.0)
                z2p = psH2.tile([Z2, HW_ * P], dt.float32, tag="z2p", bufs=2)
                nc.tensor.matmul(out=z2p[:, 0:cw], lhsT=W2e_t[:, 0, :],
                                 rhs=z1rT0[:, 0:cw], start=True, stop=False)
                nc.tensor.matmul(out=z2p[:, 0:cw],
                                 lhsT=W2e_t[0:HID - P + 1, 1, :],
                                 rhs=z1b[:, 0:cw], start=False, stop=True)
                z2b = z2r97[bi % 3]
                nc.scalar.activation(out=z2b[0:Z2, 0:cw], in_=z2p[:, 0:cw],
                                     func=mybir.ActivationFunctionType.Relu)
                z3p = psH2.tile([1, HW_ * P], dt.float32, tag="z3p", bufs=2)
                nc.tensor.matmul(out=z3p[:, 0:cw], lhsT=W3e_t[:],
                                 rhs=z2b[:, 0:cw], start=True, stop=True)
                z3b = sbE.tile([1, HW_ * P], dt.float32, tag="z3b", bufs=2)
                nc.scalar.copy(out=z3b[:, 0:cw], in_=z3p[:, 0:cw])
                nc.sync.dma_start(out=out_z[:, t0 * P:t0 * P + cw],
                                  in_=z3b[:, 0:cw])

            # head pass L: chunk 0 per block from local u_loc, overlapping
            # the u AllGather.
            eaL = [pers.tile([EA + 1, P], dt.float16, name=f"eaL_{i}")
                   for i in range(2)]
            for i in range(2):
                nc.vector.memset(eaL[i][:], 1.0)
            bi = 0
            _vt = ExitStack()
            _vt.enter_context(tc.tile_wait_until(200))
            for b in range(B):
                StH0 = gatU.tile([P, P], dt.float16, tag="StH0", bufs=2)
                nc.sync.dma_start(out=StH0[:], in_=Stt_p[:, b, 0:P])
                ea0 = eaL[b % 2]
                nc.sync.dma_start(out=ea0[0:EA, :],
                                  in_=eaT[:, b * K * P:b * K * P + P])
                lidx = gatU.tile([P, 8], dt.int16, tag="lidxH", bufs=2)
                nc.sync.dma_start(out=lidx[:],
                                  in_=glloc_p[:, b * 8:(b + 1) * 8])
                u0 = gatU.tile([P, 1, UW], dt.float16, tag="u0", bufs=2)
                nc.gpsimd.dma_gather(u0[:, :, :], u_loc[:, :],
                                     lidx[:, :], P, regN[P], UW)
                z1T = psH.tile([P, 2, HW_ * P], dt.float32, tag="z1T")
                for ci, (f0, fl) in enumerate(((0, P), (P, HID - P))):
                    nc.tensor.matmul(
                        out=z1T[:fl, ci, 0:P], lhsT=w_loc[:, b, f0:f0 + fl],
                        rhs=StH0[:], start=True, stop=False,
                        skip_group_check=True)
                    nc.tensor.matmul(
                        out=z1T[:fl, ci, 0:P], lhsT=W1c5_t[:, f0:f0 + fl],
                        rhs=ea0[:, :], start=False, stop=False,
                        skip_group_check=True)
                    nc.tensor.matmul(
                        out=z1T[:fl, ci, 0:P], lhsT=u0[:, 0, f0:f0 + fl],
                        rhs=ident16[:], start=False, stop=True,
                        skip_group_check=True)
                head_tail(z1T, P, b * K, bi)
                bi += 1

            _vt.close()
            # head pass R: remote chunks 1..K-1 after the u AllGather.
            _vt = ExitStack()
            _vt.enter_context(tc.tile_wait_until(210))
            for b in range(B):
                StH = gatU.tile([P, K, P], dt.float16, tag="StH", bufs=2)
                nc.sync.dma_start(out=StH[:].rearrange("p k j -> p (k j)"),
                                  in_=Stt_p[:, b, :])
                ea_t = ea5[b % 2]
                nc.sync.dma_start(out=ea_t[0:EA, :],
                                  in_=eaT[:, b * K * P:(b + 1) * K * P])
                S16 = K * P // 16
                SH16 = HIC * P // 16
                ilo = gatU.tile([P, S16], dt.int16, tag="ilo", bufs=2)
                nc.sync.dma_start(out=ilo[:], in_=gilo_p[:, b * S16:(b + 1) * S16])
                ihi = gatU.tile([P, SH16], dt.int16, tag="ihi", bufs=2)
                nc.sync.dma_start(out=ihi[:],
                                  in_=gihi_p[:, b * SH16:(b + 1) * SH16])
                uL = gatU.tile([P, K, UW], dt.float16, tag="uL", bufs=2)
                uH = gatU.tile([P, HIC, UW], dt.float16, tag="uH", bufs=2)
                c0 = 1
                while c0 < K:
                    cn = min(6, K - c0)
                    nc.gpsimd.dma_gather(
                        uL[:, c0:c0 + cn, :], u_tbl[0:32768, :],
                        ilo[:, c0 * 8:(c0 + cn) * 8],
                        cn * P, regN[cn * P], UW)
                    c0 += cn
                nc.gpsimd.dma_gather(
                    uH[:], u_tbl[32768:NpA, :], ihi[:],
                    HIC * P, regN[HIC * P], UW)
                for kk0 in range(1, K, HW_):
                    wn = min(HW_, K - kk0)
                    t0 = b * K + kk0
                    cw = wn * P
                    if kk0 + wn <= K - HIC:
                        ug = uL[:, kk0:kk0 + wn, 0:HID]
                    else:
                        ugt = sbE.tile([P, HW_, HID], dt.float16, tag="ug", bufs=2)
                        h0_ = max(kk0, K - HIC)
                        if h0_ > kk0:
                            nc.vector.tensor_copy(out=ugt[:, 0:h0_ - kk0, :],
                                                  in_=uL[:, kk0:h0_, 0:HID])
                        nc.vector.tensor_tensor(
                            out=ugt[:, h0_ - kk0:wn, :],
                            in0=uL[:, h0_:kk0 + wn, 0:HID],
                            in1=uH[:, h0_ - (K - HIC):kk0 + wn - (K - HIC), 0:HID],
                            op=mybir.AluOpType.add)
                        ug = ugt[:, 0:wn, :]
                    gsl = 0
                    z1T = psH.tile([P, 2, HW_ * P], dt.float32, tag="z1T")
                    for ci, (f0, fl) in enumerate(((0, P), (P, HID - P))):
                        nc.tensor.matmul(
                            out=z1T[:fl, ci, 0:cw], lhsT=w_loc[:, b, f0:f0 + fl],
                            rhs=StH[:, kk0:kk0 + wn, :],
                            start=True, stop=False, skip_group_check=True)
                        nc.tensor.matmul(
                            out=z1T[:fl, ci, 0:cw], lhsT=W1c5_t[:, f0:f0 + fl],
                            rhs=ea_t[:, kk0 * P:kk0 * P + cw],
                            start=False, stop=False, skip_group_check=True)
                        for w_ in range(wn):
                            nc.tensor.matmul(
                                out=z1T[:fl, ci, w_ * P:(w_ + 1) * P],
                                lhsT=ug[:, gsl + w_, f0:f0 + fl],
                                rhs=ident16[:],
                                start=False, stop=(w_ == wn - 1),
                                skip_group_check=True)
                    head_tail(z1T, cw, t0, bi)
                    bi += 1
            _vt.close()
            _head.close()
            _work.close()

    from concourse.library_overlay import lower_extended_insts
    lower_extended_insts(nc)
    return nc


# ----------------------------------------------------------------------------
# public entry
# ----------------------------------------------------------------------------

def _run(inputs, n_cores, runner):
    shards, meta = _host_prep(
        inputs["x"], inputs["edge_index"], inputs["edge_attr"], inputs["batch"],
        inputs["group_ptr"], inputs["time_group_ids"], inputs["group_probs"],
        inputs["splitter_probs"], inputs["endpoint_preds"], n_cores)
    w = _host_weights(
        inputs["group_probs"], inputs["endpoint_preds"],
        *[inputs[k] for k in [
            "W_in", "b_in", "Wq", "Wk", "Wv", "We", "Wo", "bo",
            "ln1_g", "ln1_b", "W_ff1", "b_ff1", "W_ff2", "b_ff2",
            "ln2_g", "ln2_b", "W_e1", "b_e1", "W_e2", "b_e2", "W_e3", "b_e3"]])
    nc = build_program(meta, n_cores)
    in_maps = []
    for c in range(n_cores):
        m = dict(shards[c])
        m.update(w)
        in_maps.append(m)
    results = runner(nc, in_maps)
    E = meta["E"]
    out = np.zeros((E, 1), np.float32)
    for c in range(n_cores):
        z = np.asarray(results[c]["out_z"]).reshape(-1)
        eid = meta["eid_sh"][c]
        valid = eid >= 0
        out[eid[valid], 0] = z[valid]
    return out


def kernel(**inputs):
    from concourse.bass_utils import run_bass_kernel_spmd

    n_cores = 8

    def runner(nc, in_maps):
        split_excess_waits(nc, max_waits=1)
        br = run_bass_kernel_spmd(nc, in_maps, core_ids=list(range(n_cores)))
        return br.results

    return _run(inputs, n_cores, runner)

